# revision 4
# baseline (speedup 1.0000x reference)
"""Cox partial-likelihood (DeepSurv) loss on 8 TRN2 NeuronCores.

Math: P_exp_sum[i] = sum_j P_exp[j] * (T[i] < T[j]); loss is a scalar
reduction over log(P_exp / (P_exp_sum + eps)) masked by events.

Device does the O(N^2) risk-set sum, data-parallel over rows:
core c owns i in [c*2048, (c+1)*2048). For each 128-wide j-chunk the
DVE builds mask[j, i] = (T_i < T_j) as exact {0,1} bf16 via is_lt
(fp32 compare -> ties exact), and the PE contracts over j with
stationary weights [hi(P_exp_j), lo(P_exp_j)] (bf16 hi/lo split ->
~17-bit mantissa), accumulating into PSUM over all 128 chunks.
Host does the remaining O(N) epilogue exactly in fp32.
"""

import numpy as np
import ml_dtypes

N = 16384
NCORES = 8
LI = N // NCORES          # rows per core
KC = N // 128             # 128-wide j-chunks
NB = LI // 512            # psum banks per core
EPS = 1e-6

_prog_cache = {}


def _build_program(reps=1):
    if reps in _prog_cache:
        return _prog_cache[reps]
    import concourse.bacc as bacc
    import concourse.tile as tile
    import concourse.mybir as mybir

    nc = bacc.Bacc(
        "TRN2", target_bir_lowering=False, debug=False, num_devices=NCORES
    )
    tib = nc.dram_tensor("tib", [128, LI], mybir.dt.float32, kind="ExternalInput").ap()
    tj = nc.dram_tensor("tj", [128, KC], mybir.dt.float32, kind="ExternalInput").ap()
    w = nc.dram_tensor("w", [128, 2 * KC], mybir.dt.bfloat16, kind="ExternalInput").ap()
    out = nc.dram_tensor("out", [2, LI], mybir.dt.float32, kind="ExternalOutput").ap()

    with tile.TileContext(nc) as tc:
        with (
            tc.tile_pool(name="const", bufs=1) as cpool,
            tc.tile_pool(name="mask", bufs=3) as mpool,
            tc.tile_pool(name="psum", bufs=1, space="PSUM") as ppool,
            tc.tile_pool(name="res", bufs=1) as rpool,
        ):
            tib_s = cpool.tile([128, LI], mybir.dt.float32)
            nc.sync.dma_start(tib_s[:], tib[:])
            tj_s = cpool.tile([128, KC], mybir.dt.float32)
            nc.sync.dma_start(tj_s[:], tj[:])
            w_s = cpool.tile([128, 2 * KC], mybir.dt.bfloat16)
            nc.sync.dma_start(w_s[:], w[:])

            psums = [
                ppool.tile([2, 512], mybir.dt.float32, name=f"psum{b}", tag=f"psum{b}")
                for b in range(NB)
            ]
            res = rpool.tile([2, LI], mybir.dt.float32)
            for _ in range(reps):
                for k in range(KC):
                    mask = mpool.tile([128, LI], mybir.dt.bfloat16)
                    nc.vector.tensor_scalar(
                        mask[:],
                        tib_s[:],
                        tj_s[:, k : k + 1],
                        None,
                        mybir.AluOpType.is_lt,
                    )
                    for b in range(NB):
                        nc.tensor.matmul(
                            psums[b][:],
                            w_s[:, 2 * k : 2 * k + 2],
                            mask[:, 512 * b : 512 * (b + 1)],
                            start=(k == 0),
                            stop=(k == KC - 1),
                        )
                for b in range(NB):
                    nc.vector.tensor_copy(res[:, 512 * b : 512 * (b + 1)], psums[b][:])
            nc.sync.dma_start(out[:], res[:])
    nc.compile()
    _prog_cache[reps] = nc
    return nc


def _make_in_maps(P_risk, T):
    P_exp = np.exp(P_risk.astype(np.float32))
    hi = P_exp.astype(ml_dtypes.bfloat16)
    lo = (P_exp - hi.astype(np.float32)).astype(ml_dtypes.bfloat16)
    # w[p, 2k+0] = hi[k*128+p], w[p, 2k+1] = lo[k*128+p]
    w = np.empty((128, 2 * KC), dtype=ml_dtypes.bfloat16)
    w[:, 0::2] = hi.reshape(KC, 128).T
    w[:, 1::2] = lo.reshape(KC, 128).T
    tjv = np.ascontiguousarray(T.astype(np.float32).reshape(KC, 128).T)
    in_maps = []
    for c in range(NCORES):
        tib = np.ascontiguousarray(
            np.broadcast_to(T[c * LI : (c + 1) * LI].astype(np.float32), (128, LI))
        )
        in_maps.append({"tib": tib, "tj": tjv, "w": w})
    return in_maps, P_exp


def _epilogue(P_risk, T, E, P_exp, P_exp_sum):
    T = T.astype(np.float32)
    has_risk = (T < T.max()).astype(np.float32)
    Ef = E.astype(np.float32) * has_risk
    P_tmp = P_exp / (P_exp_sum + np.float32(EPS))
    upper = P_tmp.max()
    P_clipped = np.clip(P_tmp, np.float32(EPS), upper)
    loss = -np.sum(np.log(P_clipped) * Ef, dtype=np.float32) / np.sum(
        Ef, dtype=np.float32
    )
    return np.asarray(loss, dtype=np.float32)


def kernel(P_risk, T, E):
    from concourse.bass_utils import run_bass_kernel_spmd

    nc = _build_program()
    in_maps, P_exp = _make_in_maps(P_risk, T)
    res = run_bass_kernel_spmd(nc, in_maps, core_ids=list(range(NCORES)))
    outs = np.stack([res.results[c]["out"] for c in range(NCORES)])  # [8, 2, LI]
    P_exp_sum = (outs[:, 0, :] + outs[:, 1, :]).reshape(N)
    return _epilogue(P_risk, T, E, P_exp, P_exp_sum)


# revision 5
# speedup vs baseline: 457.2011x; 457.2011x over previous
"""Cox partial-likelihood (DeepSurv) loss on 8 TRN2 NeuronCores.

Math: P_exp_sum[i] = sum_j P_exp[j] * (T[i] < T[j]); loss is a scalar
reduction over log(P_exp / (P_exp_sum + eps)) masked by events.

Device does the O(N^2) risk-set sum, data-parallel over rows:
core c owns i in [c*2048, (c+1)*2048). For each 128-wide j-chunk the
DVE builds mask[j, i] = (T_i < T_j) as exact {0,1} bf16 via is_lt
(fp32 compare -> ties exact), and the PE contracts over j with
stationary weights [hi(P_exp_j), lo(P_exp_j)] (bf16 hi/lo split ->
~17-bit mantissa), accumulating into PSUM over all 128 chunks.
Host does the remaining O(N) epilogue exactly in fp32.
"""

import numpy as np
import ml_dtypes

N = 16384
NCORES = 8
LI = N // NCORES          # rows per core
KC = N // 128             # 128-wide j-chunks
NB = LI // 512            # psum banks per core
EPS = 1e-6

_prog_cache = {}


def _build_program(reps=1):
    if reps in _prog_cache:
        return _prog_cache[reps]
    import concourse.bacc as bacc
    import concourse.tile as tile
    import concourse.mybir as mybir

    nc = bacc.Bacc(
        "TRN2", target_bir_lowering=False, debug=False, num_devices=NCORES
    )
    tib = nc.dram_tensor("tib", [128, LI], mybir.dt.float32, kind="ExternalInput").ap()
    tj = nc.dram_tensor("tj", [128, KC], mybir.dt.float32, kind="ExternalInput").ap()
    w = nc.dram_tensor("w", [128, 2 * KC], mybir.dt.bfloat16, kind="ExternalInput").ap()
    out = nc.dram_tensor("out", [2, LI], mybir.dt.float32, kind="ExternalOutput").ap()

    with tile.TileContext(nc) as tc:
        with (
            tc.tile_pool(name="const", bufs=1) as cpool,
            tc.tile_pool(name="mask", bufs=32) as mpool,
            tc.tile_pool(name="psum", bufs=1, space="PSUM") as ppool,
            tc.tile_pool(name="res", bufs=1) as rpool,
        ):
            tib_s = cpool.tile([128, LI], mybir.dt.float32)
            nc.sync.dma_start(tib_s[:], tib[:])
            tj_s = cpool.tile([128, KC], mybir.dt.float32)
            nc.sync.dma_start(tj_s[:], tj[:])
            w_s = cpool.tile([128, 2 * KC], mybir.dt.bfloat16)
            nc.sync.dma_start(w_s[:], w[:])

            psums = [
                ppool.tile([2, 512], mybir.dt.float32, name=f"psum{b}", tag=f"psum{b}")
                for b in range(NB)
            ]
            res = rpool.tile([2, LI], mybir.dt.float32)
            for _ in range(reps):
                for k in range(KC):
                    mask = mpool.tile([128, LI], mybir.dt.bfloat16)
                    nc.vector.tensor_scalar(
                        mask[:],
                        tib_s[:],
                        tj_s[:, k : k + 1],
                        None,
                        mybir.AluOpType.is_lt,
                    )
                    for b in range(NB):
                        nc.tensor.matmul(
                            psums[b][:],
                            w_s[:, 2 * k : 2 * k + 2],
                            mask[:, 512 * b : 512 * (b + 1)],
                            start=(k == 0),
                            stop=(k == KC - 1),
                        )
                for b in range(NB):
                    nc.vector.tensor_copy(res[:, 512 * b : 512 * (b + 1)], psums[b][:])
            nc.sync.dma_start(out[:], res[:])
    nc.compile()
    _prog_cache[reps] = nc
    return nc


def _make_in_maps(P_risk, T):
    P_exp = np.exp(P_risk.astype(np.float32))
    hi = P_exp.astype(ml_dtypes.bfloat16)
    lo = (P_exp - hi.astype(np.float32)).astype(ml_dtypes.bfloat16)
    # w[p, 2k+0] = hi[k*128+p], w[p, 2k+1] = lo[k*128+p]
    w = np.empty((128, 2 * KC), dtype=ml_dtypes.bfloat16)
    w[:, 0::2] = hi.reshape(KC, 128).T
    w[:, 1::2] = lo.reshape(KC, 128).T
    tjv = np.ascontiguousarray(T.astype(np.float32).reshape(KC, 128).T)
    in_maps = []
    for c in range(NCORES):
        tib = np.ascontiguousarray(
            np.broadcast_to(T[c * LI : (c + 1) * LI].astype(np.float32), (128, LI))
        )
        in_maps.append({"tib": tib, "tj": tjv, "w": w})
    return in_maps, P_exp


def _epilogue(P_risk, T, E, P_exp, P_exp_sum):
    T = T.astype(np.float32)
    has_risk = (T < T.max()).astype(np.float32)
    Ef = E.astype(np.float32) * has_risk
    P_tmp = P_exp / (P_exp_sum + np.float32(EPS))
    upper = P_tmp.max()
    P_clipped = np.clip(P_tmp, np.float32(EPS), upper)
    loss = -np.sum(np.log(P_clipped) * Ef, dtype=np.float32) / np.sum(
        Ef, dtype=np.float32
    )
    return np.asarray(loss, dtype=np.float32)


def kernel(P_risk, T, E):
    from concourse.bass_utils import run_bass_kernel_spmd

    nc = _build_program()
    in_maps, P_exp = _make_in_maps(P_risk, T)
    res = run_bass_kernel_spmd(nc, in_maps, core_ids=list(range(NCORES)))
    outs = np.stack([res.results[c]["out"] for c in range(NCORES)])  # [8, 2, LI]
    P_exp_sum = (outs[:, 0, :] + outs[:, 1, :]).reshape(N)
    return _epilogue(P_risk, T, E, P_exp, P_exp_sum)


# revision 6
# speedup vs baseline: 578.5267x; 1.2654x over previous
"""Cox partial-likelihood (DeepSurv) loss on 8 TRN2 NeuronCores.

Math: P_exp_sum[i] = sum_j P_exp[j] * (T[i] < T[j]); loss is a scalar
reduction over log(P_exp / (P_exp_sum + eps)) masked by events.

Device does the O(N^2) risk-set sum, data-parallel over rows:
core c owns i in [c*2048, (c+1)*2048). For each 128-wide j-chunk the
DVE builds mask[j, i] = (T_i < T_j) as exact {0,1} bf16 via is_lt
(fp32 compare -> ties exact), and the PE contracts over j with
stationary weights [hi(P_exp_j), lo(P_exp_j)] (bf16 hi/lo split ->
~17-bit mantissa), accumulating into PSUM over all 128 chunks.
Host does the remaining O(N) epilogue exactly in fp32.
"""

import numpy as np
import ml_dtypes

N = 16384
NCORES = 8
LI = N // NCORES          # rows per core
KC = N // 128             # 128-wide j-chunks
NB = LI // 512            # psum banks per core
EPS = 1e-6

_prog_cache = {}


def _build_program(reps=1):
    if reps in _prog_cache:
        return _prog_cache[reps]
    import concourse.bacc as bacc
    import concourse.tile as tile
    import concourse.mybir as mybir

    nc = bacc.Bacc(
        "TRN2", target_bir_lowering=False, debug=False, num_devices=NCORES
    )
    tib = nc.dram_tensor("tib", [128, LI], mybir.dt.float32, kind="ExternalInput").ap()
    tj = nc.dram_tensor("tj", [128, KC], mybir.dt.float32, kind="ExternalInput").ap()
    w = nc.dram_tensor("w", [128, 2 * KC], mybir.dt.bfloat16, kind="ExternalInput").ap()
    out = nc.dram_tensor("out", [2, LI], mybir.dt.float32, kind="ExternalOutput").ap()

    with tile.TileContext(nc) as tc:
        with (
            tc.tile_pool(name="const", bufs=1) as cpool,
            tc.tile_pool(name="mask", bufs=32) as mpool,
            tc.tile_pool(name="psum", bufs=1, space="PSUM") as ppool,
            tc.tile_pool(name="res", bufs=1) as rpool,
        ):
            tib_s = cpool.tile([128, LI], mybir.dt.float32)
            nc.sync.dma_start(tib_s[:], tib[:])
            tj_s = cpool.tile([128, KC], mybir.dt.float32)
            nc.sync.dma_start(tj_s[:], tj[:])
            w_s = cpool.tile([128, 2 * KC], mybir.dt.bfloat16)
            nc.sync.dma_start(w_s[:], w[:])

            psums = [
                ppool.tile([2, 512], mybir.dt.float32, name=f"psum{b}", tag=f"psum{b}")
                for b in range(NB)
            ]
            res = rpool.tile([2, LI], mybir.dt.float32)
            for _ in range(reps):
                for k in range(KC):
                    mask = mpool.tile([128, LI], mybir.dt.bfloat16)
                    nc.vector.tensor_scalar(
                        mask[:],
                        tib_s[:],
                        tj_s[:, k : k + 1],
                        None,
                        mybir.AluOpType.is_lt,
                    )
                    for b in range(NB):
                        nc.tensor.matmul(
                            psums[b][:],
                            w_s[:, 2 * k : 2 * k + 2],
                            mask[:, 512 * b : 512 * (b + 1)],
                            start=(k == 0),
                            stop=(k == KC - 1),
                        )
                for b in range(NB):
                    nc.vector.tensor_copy(res[:, 512 * b : 512 * (b + 1)], psums[b][:])
            nc.sync.dma_start(out[:], res[:])
    nc.compile()
    _prog_cache[reps] = nc
    return nc


def _make_in_maps(P_risk, T):
    P_exp = np.exp(P_risk.astype(np.float32))
    hi = P_exp.astype(ml_dtypes.bfloat16)
    lo = (P_exp - hi.astype(np.float32)).astype(ml_dtypes.bfloat16)
    # w[p, 2k+0] = hi[k*128+p], w[p, 2k+1] = lo[k*128+p]
    w = np.empty((128, 2 * KC), dtype=ml_dtypes.bfloat16)
    w[:, 0::2] = hi.reshape(KC, 128).T
    w[:, 1::2] = lo.reshape(KC, 128).T
    tjv = np.ascontiguousarray(T.astype(np.float32).reshape(KC, 128).T)
    in_maps = []
    for c in range(NCORES):
        tib = np.ascontiguousarray(
            np.broadcast_to(T[c * LI : (c + 1) * LI].astype(np.float32), (128, LI))
        )
        in_maps.append({"tib": tib, "tj": tjv, "w": w})
    return in_maps, P_exp


def _epilogue(P_risk, T, E, P_exp, P_exp_sum):
    T = T.astype(np.float32)
    has_risk = (T < T.max()).astype(np.float32)
    Ef = E.astype(np.float32) * has_risk
    P_tmp = P_exp / (P_exp_sum + np.float32(EPS))
    upper = P_tmp.max()
    P_clipped = np.clip(P_tmp, np.float32(EPS), upper)
    loss = -np.sum(np.log(P_clipped) * Ef, dtype=np.float32) / np.sum(
        Ef, dtype=np.float32
    )
    return np.asarray(loss, dtype=np.float32)


def kernel(P_risk, T, E):
    from concourse.bass_utils import run_bass_kernel_spmd

    nc = _build_program()
    in_maps, P_exp = _make_in_maps(P_risk, T)
    S_total = float(P_exp.sum(dtype=np.float64))
    last_err = None
    for _attempt in range(3):
        try:
            res = run_bass_kernel_spmd(nc, in_maps, core_ids=list(range(NCORES)))
            outs = np.stack([res.results[c]["out"] for c in range(NCORES)])
            P_exp_sum = (outs[:, 0, :] + outs[:, 1, :]).reshape(N)
            # sanity: each risk-set sum lies in [0, sum(P_exp)]; the row
            # holding max(T) has an empty risk set. Guards against a
            # silently-failed device execution.
            ok = (
                np.isfinite(P_exp_sum).all()
                and float(P_exp_sum.min()) >= -1e-2
                and float(P_exp_sum.max()) <= S_total * 1.001
                and abs(float(P_exp_sum[int(np.argmax(T))])) < 1e-2
                and float(P_exp_sum.max()) > 0.0
            )
            if ok:
                return _epilogue(P_risk, T, E, P_exp, P_exp_sum)
            last_err = RuntimeError("device output failed sanity check")
        except Exception as e:  # transient NRT device errors happen
            last_err = e
    raise last_err
